# revision 25
# baseline (speedup 1.0000x reference)
"""Additive (Bahdanau) attention on 8 Trainium2 NeuronCores.

reference:
    query_proj = query @ W2                               [B, H]
    keys_proj  = einsum('bsd,dh->bsh', values, W1)        [B, S, H]
    energy     = tanh(query_proj[:, None, :] + keys_proj) [B, S, H]
    scores     = einsum('bsh,h->bs', energy, v)           [B, S]
    attn       = softmax(scores, -1)                      [B, S]
    context    = einsum('bs,bsd->bd', attn, values)       [B, D]
    returns (context, attn)

Sharding: pure data-parallel over batch B=32 -> 4 batches per core, no
collectives. Per core, per batch, per s-block of 1024 (resident in SBUF):

  keys_proj^T tiles [h=128p, s=512] = W1_chunk.T @ valuesT_tile (fp32r
  matmuls, full-rate at N=512), tanh fused with the per-partition bias
  query_proj column on ScalarE, v-dot via K=128/M=1 matmuls accumulating
  scores [1, 512] in PSUM, exp (no max subtraction needed: |scores|<60,
  constant -20 bias for margin) with the softmax denominator via ACT
  accum_out, e broadcast across partitions via a K=1 ones-matmul into
  PSUM, and the context contraction sum_s e_s * valuesT[d, s] as DVE
  tensor_tensor_reduce against the SBUF-resident valuesT block.
  Context is normalized by 1/Z on device; attention rows ship as raw
  e plus Z and are divided on the host.
"""

import sys
import types

import numpy as np

B, S, D, H = 32, 2048, 1024, 1024
NCORES = 8
BP = B // NCORES  # batches per core
PD = 128
DC = D // PD  # 8 contraction chunks
HC = H // PD  # 8 hidden chunks
TS = 512  # psum s-tile (matmul moving dim)
SBLK = 1024  # SBUF-resident s-block
NBLK = S // SBLK
NST = SBLK // TS
NCHUNK = NBLK * NST  # exp chunks per batch
TAILOFF = D + S + 8
OUTW = TAILOFF + D  # ctx | raw e | Z | last-chunk ctx row (host-merged)
EXP_BIAS = -20.0  # constant shift inside exp; cancels in softmax

_CACHE = {}
ONES_HOST = np.ones((1, PD), dtype=np.float32)


def _install_axon_shim():
    """Recreate antenv.axon_hooks (absent in this image) and register the
    ctypes NTFF profiling hook so trace=True works under axon."""
    if "antenv.axon_hooks" in sys.modules:
        return
    try:
        import antenv
    except ImportError:
        return
    mod = types.ModuleType("antenv.axon_hooks")
    _state = {"hook": None}
    mod.set_axon_ntff_profile_hook = lambda h: _state.__setitem__("hook", h)
    mod.get_axon_ntff_profile_hook = lambda: _state["hook"]
    sys.modules["antenv.axon_hooks"] = mod
    antenv.axon_hooks = mod
    try:
        from trn_agent_boot.trn_boot import _ntff_profile_via_ctypes

        hook = _ntff_profile_via_ctypes("/opt/axon/libaxon_pjrt.so")
        if hook is not None:
            mod.set_axon_ntff_profile_hook(hook)
    except Exception:
        pass


def _build():
    import concourse.bacc as bacc
    import concourse.mybir as mybir
    from concourse.tile import TileContext
    from concourse import masks

    F32 = mybir.dt.float32
    F32R = mybir.dt.float32r
    AF = mybir.ActivationFunctionType
    ALU = mybir.AluOpType

    nc = bacc.Bacc("TRN2", target_bir_lowering=False)
    vT = nc.declare_dram_parameter("valuesT", [BP, D, S], F32R, isOutput=False)
    w1 = nc.declare_dram_parameter("w1", [HC, PD, DC, PD], F32R, isOutput=False)
    w2 = nc.declare_dram_parameter("w2", [D, H], F32R, isOutput=False)
    qT = nc.declare_dram_parameter("qT", [D, BP], F32R, isOutput=False)
    vv = nc.declare_dram_parameter("v", [H, 1], F32R, isOutput=False)
    ones_in = nc.declare_dram_parameter("ones", [1, PD], F32R, isOutput=False)
    vnat = nc.declare_dram_parameter("vnat", [TS // PD, PD, D], F32R, isOutput=False)
    out = nc.declare_dram_parameter("out", [BP, OUTW], F32, isOutput=True)

    with TileContext(nc) as tc:
        with (
            tc.tile_pool(name="const", bufs=1) as cpool,
            tc.tile_pool(name="vt", bufs=3) as vtpool,
            tc.tile_pool(name="en", bufs=4) as enpool,
            tc.tile_pool(name="erow", bufs=2) as epool,
            tc.tile_pool(name="kp", bufs=3, space="PSUM") as kpool,
            tc.tile_pool(name="sc", bufs=2, space="PSUM") as scpool,
            tc.tile_pool(name="bc", bufs=2, space="PSUM") as bpool,
        ):
            w1_sb = cpool.tile([PD, HC, DC, PD], F32R, tag="w1")
            w2_sb = cpool.tile([PD, DC, H], F32R, tag="w2")
            qt_sb = cpool.tile([PD, DC, BP], F32R, tag="qt")
            v_sb = cpool.tile([PD, HC], F32R, tag="v")
            ones = cpool.tile([1, PD], F32R, tag="ones")
            onesf = cpool.tile([1, PD], F32, tag="onesf")
            ident = cpool.tile([PD, PD], F32, tag="ident")
            qp_sb = cpool.tile([PD, HC, BP], F32, tag="qp")
            qprow_sb = cpool.tile([BP, H], F32, tag="qprow")
            z_sb = cpool.tile([1, BP, NCHUNK], F32, tag="z")
            zt_sb = cpool.tile([1, BP], F32, tag="zt")
            rz_sb = cpool.tile([1, BP], F32, tag="rz")
            rzc_sb = cpool.tile([PD, BP], F32, tag="rzc")
            ctx_sb = cpool.tile([PD, BP, DC, NCHUNK], F32, tag="ctx")
            ctxf_sb = cpool.tile([PD, BP, DC], F32, tag="ctxf")
            ctxs_sb = cpool.tile([PD, BP, DC], F32, tag="ctxs")
            ct_sb = cpool.tile([DC, BP, PD], F32, tag="ct")
            dump = cpool.tile([PD, TS], F32, tag="dump")
            ebias = cpool.tile([1, 1], F32, tag="ebias")
            ecol_sb = cpool.tile([PD, TS // PD], F32R, tag="ecol")
            crow_sb = cpool.tile([1, D], F32, tag="crow")

            nc.vector.memset(onesf[:], 1.0)
            nc.vector.memset(ebias[:], EXP_BIAS)
            masks.make_identity(nc, ident[:])
            vt_tiles = {}
            vt00 = vtpool.tile([PD, DC, SBLK], F32R, tag="vt", name="vt")
            vt_tiles[(0, 0)] = vt00
            # DMA issue order tracks first consumption: w2/qT feed qp, then
            # the first keys group needs vt00's st0 half and w1's hc0 slab
            # (w1 arrives pre-shuffled hc-major so one 512KB DMA per hc).
            def _vt_dma(tile, dc, bi_blk, c0, c1):
                eng = nc.sync
                bi, blk = bi_blk if isinstance(bi_blk, tuple) else (0, 0)
                eng.dma_start(
                    out=tile[:, dc, c0:c1],
                    in_=vT[bi, dc * PD:(dc + 1) * PD, blk * SBLK + c0:blk * SBLK + c1],
                )

            def _w1_half(hc, h):
                half = DC // 2
                nc.sync.dma_start(
                    out=w1_sb[:, hc, h * half:(h + 1) * half, :],
                    in_=w1[hc, :, h * half:(h + 1) * half, :],
                )

            nc.sync.dma_start(
                out=qt_sb[:],
                in_=qT.rearrange("(c p) b -> p c b", p=PD),
            )
            # interleave so w2-half0 (qp), vt00-st0 and the first w1 slabs
            # stream on distinct queues concurrently: per-queue bandwidth
            # (~21 GB/s) makes large single-queue chunks the head bottleneck
            for dc in range(DC):
                nc.sync.dma_start(out=w2_sb[:, dc, 0:TS], in_=w2[dc * PD:(dc + 1) * PD, 0:TS])
                _vt_dma(vt00, dc, 0, 0, TS)
                if dc % 2 == 1:
                    _w1_half(dc // 2, 0)
                    _w1_half(dc // 2, 1)
            for dc in range(DC):
                nc.sync.dma_start(out=w2_sb[:, dc, TS:H], in_=w2[dc * PD:(dc + 1) * PD, TS:H])
                if dc % 2 == 1:
                    _w1_half(4 + dc // 2, 0)
                    _w1_half(4 + dc // 2, 1)
                _vt_dma(vt00, dc, 0, TS, SBLK)
                if dc == 5:
                    nc.sync.dma_start(out=ones[:], in_=ones_in[:])
                    nc.sync.dma_start(
                        out=v_sb[:], in_=vv.rearrange("(c p) o -> p (c o)", p=PD)
                    )

            # query_proj rows via qT as the (4-column) stationary operand,
            # then PE-transpose into bias columns. The second half is
            # deferred into the group loop so the first keys groups run
            # while w2's second half is still arriving.
            HPT = TS // PD  # transposed hc columns per half

            def qp_half(hh):
                qprow_ps = kpool.tile([BP, TS], F32, tag="kp", name="qprow_ps")
                for dc in range(DC):
                    nc.tensor.matmul(
                        qprow_ps[:],
                        qt_sb[:, dc, :],
                        w2_sb[:, dc, hh * TS:(hh + 1) * TS],
                        start=(dc == 0),
                        stop=(dc == DC - 1),
                    )
                nc.vector.tensor_copy(qprow_sb[0:BP, hh * TS:(hh + 1) * TS], qprow_ps[:])
                for hc in range(hh * HPT, (hh + 1) * HPT):
                    qpcol_ps = kpool.tile([PD, BP], F32, tag="kp", name="qpcol_ps")
                    nc.tensor.transpose(
                        qpcol_ps[:], qprow_sb[0:BP, hc * PD:(hc + 1) * PD], ident[0:BP, 0:BP]
                    )
                    nc.vector.tensor_copy(qp_sb[:, hc, :], qpcol_ps[:])

            qp_half(0)

            groups = [
                (bi, blk, st, hc)
                for bi in range(BP)
                for blk in range(NBLK)
                for st in range(NST)
                for hc in range(HC)
            ]
            NG = len(groups)
            en_tiles = {}
            vnat_tiles = []
            sc_tiles = {}
            bc_tiles = {}
            erows = {}
            deferred = {}

            def defer(pos, fn):
                deferred.setdefault(pos, []).append(fn)

            def emit_bc(pos, bj, blkj, stj):
                erow = erows[bj]
                off = blkj * SBLK + stj * TS
                final = bj == BP - 1 and blkj == NBLK - 1 and stj == NST - 1
                if final:
                    # final chunk: PE is idle by now, so contract this
                    # half-block against a natural-layout values slice and
                    # ship the resulting row for the host to merge
                    nc.vector.tensor_reduce(
                        zt_sb[0:1, bj:bj + 1], z_sb[0:1, bj, :],
                        axis=mybir.AxisListType.X, op=ALU.add,
                    )
                    nc.vector.reciprocal(rz_sb[0:1, bj:bj + 1], zt_sb[0:1, bj:bj + 1])
                    nc.sync.dma_start(out=out[bj:bj + 1, D:D + S], in_=erow[0:1, :].bitcast(F32))
                    nc.sync.dma_start(out=out[bj:bj + 1, D + S:D + S + 1], in_=zt_sb[0:1, bj:bj + 1])
                    for k in range(TS // PD):
                        ecol_ps = scpool.tile([PD, 1], F32, tag="sc", name="ecol_ps")
                        nc.tensor.transpose(
                            ecol_ps[:],
                            erow[0:1, off + k * PD:off + (k + 1) * PD].bitcast(F32),
                            onesf[0:1, 0:1],
                        )
                        nc.scalar.copy(ecol_sb[:, k:k + 1], ecol_ps[:])
                    vns = vnat_tiles[0]
                    for dh in range(D // TS):
                        crow_ps = scpool.tile([1, TS], F32, tag="sc", name="crow_ps")
                        for k in range(TS // PD):
                            nc.tensor.matmul(
                                crow_ps[:],
                                ecol_sb[:, k:k + 1],
                                vns[:, k, dh * TS:(dh + 1) * TS],
                                start=(k == 0),
                                stop=(k == TS // PD - 1),
                            )
                        nc.vector.tensor_scalar_mul(
                            crow_sb[0:1, dh * TS:(dh + 1) * TS], crow_ps[:],
                            rz_sb[0:1, bj:bj + 1],
                        )
                    nc.sync.dma_start(
                        out=out[bj:bj + 1, TAILOFF:TAILOFF + D], in_=crow_sb[0:1, :]
                    )
                    defer(pos + 2, lambda: rz_bcast(bj))
                    defer(pos + 4, lambda: ctx_finish(bj))
                    defer(pos + 6, lambda: transpose_out(bj))
                    return
                bc = bpool.tile([PD, TS], F32, tag="bc", name="bc")
                nc.tensor.matmul(
                    bc[:],
                    ones[:],
                    erow[0:1, off:off + TS],
                    start=True,
                    stop=True,
                )
                last = stj == NST - 1 and blkj == NBLK - 1
                if last:
                    # softmax denominator first: keeps the deferred PE
                    # rz-broadcast from stalling behind the ctx chain
                    nc.vector.tensor_reduce(
                        zt_sb[0:1, bj:bj + 1], z_sb[0:1, bj, :],
                        axis=mybir.AxisListType.X, op=ALU.add,
                    )
                    nc.vector.reciprocal(rz_sb[0:1, bj:bj + 1], zt_sb[0:1, bj:bj + 1])
                    nc.sync.dma_start(out=out[bj:bj + 1, D:D + S], in_=erow[0:1, :].bitcast(F32))
                    nc.sync.dma_start(out=out[bj:bj + 1, D + S:D + S + 1], in_=zt_sb[0:1, bj:bj + 1])
                # context partial sums for this half-block on DVE
                vt = vt_tiles[(bj, blkj)]
                for dc in range(DC):
                    nc.vector.tensor_tensor(
                        out=dump[:],
                        in0=vt[:, dc, stj * TS:(stj + 1) * TS].bitcast(F32),
                        in1=bc[:],
                        op=ALU.mult,
                    )
                    nc.vector.tensor_reduce(
                        ctx_sb[:, bj, dc, blkj * NST + stj:blkj * NST + stj + 1],
                        dump[:],
                        axis=mybir.AxisListType.X,
                        op=ALU.add,
                    )
                if last:
                    defer(pos + 2, lambda: rz_bcast(bj))
                    defer(pos + 8, lambda: ctx_finish(bj))
                    defer(pos + 12, lambda: transpose_out(bj))

            def rz_bcast(bj):
                rz_ps = scpool.tile([PD, 1], F32, tag="sc", name="rz_ps")
                nc.tensor.matmul(
                    rz_ps[:], onesf[:], rz_sb[0:1, bj:bj + 1], start=True, stop=True
                )
                nc.vector.tensor_copy(rzc_sb[:, bj:bj + 1], rz_ps[:])

            def ctx_finish(bj):
                nparts = NCHUNK - 1 if bj == BP - 1 else NCHUNK
                nc.vector.tensor_reduce(
                    ctxf_sb[:, bj, :], ctx_sb[:, bj, :, 0:nparts],
                    axis=mybir.AxisListType.X, op=ALU.add,
                )
                nc.vector.tensor_scalar_mul(
                    ctxs_sb[:, bj, :], ctxf_sb[:, bj, :], rzc_sb[:, bj:bj + 1]
                )

            def transpose_out(bj):
                ct_ps = scpool.tile([DC, PD], F32, tag="sc", name="ct_ps")
                nc.tensor.transpose(ct_ps[:], ctxs_sb[:, bj, :], ident[:])
                nc.vector.tensor_copy(ct_sb[:, bj, :], ct_ps[:])
                nc.sync.dma_start(
                    out=out[bj:bj + 1, 0:D].rearrange("a (b c) -> (a b) c", b=DC),
                    in_=ct_sb[:, bj, :],
                )

            def emit_vdot(pos, j):
                bj, blkj, stj, hcj = groups[j]
                scj = sc_tiles[(bj, blkj, stj)]
                nc.tensor.matmul(
                    scj[:],
                    v_sb[:, hcj:hcj + 1],
                    en_tiles.pop(j)[:],
                    start=(hcj == 0),
                    stop=(hcj == HC - 1),
                )
                if hcj == HC - 1:
                    # chunk finished: exp with Z-partial via accum_out
                    if (blkj, stj) == (0, 0):
                        erows[bj] = epool.tile([1, S], F32R, tag="erow", name="erow")
                    erow = erows[bj]
                    c = blkj * NST + stj
                    off = blkj * SBLK + stj * TS
                    nc.scalar.activation(
                        erow[0:1, off:off + TS],
                        scj[:],
                        AF.Exp,
                        bias=ebias[0:1, :],
                        scale=1.0,
                        accum_out=z_sb[0:1, bj, c:c + 1],
                    )
                    defer(pos + 1, lambda: emit_bc(pos + 1, bj, blkj, stj))

            for i, (bi, blk, st, hc) in enumerate(groups):
                if st == 0 and hc == 0 and (bi, blk) not in vt_tiles:
                    vt = vtpool.tile([PD, DC, SBLK], F32R, tag="vt", name="vt")
                    vt_tiles[(bi, blk)] = vt
                    for dc in range(DC):
                        _vt_dma(vt, dc, (bi, blk), 0, SBLK)
                    if (bi, blk) == (BP - 1, NBLK - 1):
                        vnat_big = vtpool.tile([PD, DC, SBLK], F32R, tag="vt", name="vnat")
                        vnat_sb = vnat_big[:, 0:TS // PD, :]
                        vnat_tiles.append(vnat_sb)
                        for k in range(TS // PD):
                            nc.sync.dma_start(out=vnat_sb[:, k, :], in_=vnat[k])
                if hc == 0:
                    sc_tiles[(bi, blk, st)] = scpool.tile([1, TS], F32, tag="sc", name="sc")
                vt = vt_tiles[(bi, blk)]
                kp = kpool.tile([PD, TS], F32, tag="kp", name="kp")
                for dc in range(DC):
                    nc.tensor.matmul(
                        kp[:],
                        w1_sb[:, hc, dc, :],
                        vt[:, dc, st * TS:(st + 1) * TS],
                        start=(dc == 0),
                        stop=(dc == DC - 1),
                    )
                en = enpool.tile([PD, TS], F32R, tag="en", name="en")
                en_tiles[i] = en
                nc.scalar.activation(
                    en[:], kp[:], AF.Tanh,
                    bias=qp_sb[:, hc, bi:bi + 1], scale=1.0,
                )
                if i == 2:
                    qp_half(1)
                for fn in deferred.pop(i, []):
                    fn()
                if i >= 2:
                    emit_vdot(i, i - 2)
                if i == NG - 1:
                    emit_vdot(i, i - 1)
                    emit_vdot(i, i)
                    while deferred:
                        j = min(deferred)
                        for fn in deferred.pop(j):
                            fn()

    nc.finalize()
    return nc


def _get_nc():
    if "nc" not in _CACHE:
        _install_axon_shim()
        _CACHE["nc"] = _build()
    return _CACHE["nc"]


def _shard_inputs(query, values, W1, W2, v):
    q = np.asarray(query, dtype=np.float32)
    vals = np.asarray(values, dtype=np.float32)
    w1 = np.asarray(W1, dtype=np.float32)
    w1 = np.ascontiguousarray(
        w1.reshape(DC, PD, HC, PD).transpose(2, 1, 0, 3)
    )
    w2 = np.ascontiguousarray(np.asarray(W2, dtype=np.float32))
    vcol = np.ascontiguousarray(np.asarray(v, dtype=np.float32).reshape(H, 1))
    in_maps = []
    for i in range(NCORES):
        sl = slice(i * BP, (i + 1) * BP)
        in_maps.append({
            "valuesT": np.ascontiguousarray(vals[sl].transpose(0, 2, 1)),
            "w1": w1,
            "w2": w2,
            "qT": np.ascontiguousarray(q[sl].T),
            "v": vcol,
            "ones": ONES_HOST,
            "vnat": np.ascontiguousarray(
                vals[i * BP + BP - 1, S - TS:S, :].reshape(TS // PD, PD, D)
            ),
        })
    return in_maps


def _run(inputs, trace=False):
    from concourse.bass_utils import run_bass_kernel_spmd

    nc = _get_nc()
    in_maps = _shard_inputs(**inputs)
    res = run_bass_kernel_spmd(nc, in_maps, list(range(NCORES)), trace=trace)
    full = np.concatenate([res.results[i]["out"] for i in range(NCORES)], axis=0)
    context = full[:, :D].copy()
    context[BP - 1::BP] += full[BP - 1::BP, TAILOFF:TAILOFF + D]
    attn = full[:, D:D + S] / full[:, D + S:D + S + 1]
    return (
        np.ascontiguousarray(context.astype(np.float32)),
        np.ascontiguousarray(attn.astype(np.float32)),
    ), res


def kernel(query, values, W1, W2, v):
    (context, attn), _ = _run(
        dict(query=query, values=values, W1=W1, W2=W2, v=v), trace=False
    )
    return context, attn


# revision 26
# speedup vs baseline: 1.0237x; 1.0237x over previous
"""Additive (Bahdanau) attention on 8 Trainium2 NeuronCores.

reference:
    query_proj = query @ W2                               [B, H]
    keys_proj  = einsum('bsd,dh->bsh', values, W1)        [B, S, H]
    energy     = tanh(query_proj[:, None, :] + keys_proj) [B, S, H]
    scores     = einsum('bsh,h->bs', energy, v)           [B, S]
    attn       = softmax(scores, -1)                      [B, S]
    context    = einsum('bs,bsd->bd', attn, values)       [B, D]
    returns (context, attn)

Sharding: pure data-parallel over batch B=32 -> 4 batches per core, no
collectives. Per core, per batch, per s-block of 1024 (resident in SBUF):

  keys_proj^T tiles [h=128p, s=512] = W1_chunk.T @ valuesT_tile (fp32r
  matmuls, full-rate at N=512), tanh fused with the per-partition bias
  query_proj column on ScalarE, v-dot via K=128/M=1 matmuls accumulating
  scores [1, 512] in PSUM, exp (no max subtraction needed: |scores|<60,
  constant -20 bias for margin) with the softmax denominator via ACT
  accum_out, e broadcast across partitions via a K=1 ones-matmul into
  PSUM, and the context contraction sum_s e_s * valuesT[d, s] as DVE
  tensor_tensor_reduce against the SBUF-resident valuesT block.
  Context is normalized by 1/Z on device; attention rows ship as raw
  e plus Z and are divided on the host.
"""

import sys
import types

import numpy as np

B, S, D, H = 32, 2048, 1024, 1024
NCORES = 8
BP = B // NCORES  # batches per core
PD = 128
DC = D // PD  # 8 contraction chunks
HC = H // PD  # 8 hidden chunks
TS = 512  # psum s-tile (matmul moving dim)
SBLK = 1024  # SBUF-resident s-block
NBLK = S // SBLK
NST = SBLK // TS
NCHUNK = NBLK * NST  # exp chunks per batch
TAILOFF = D + S + 8
OUTW = TAILOFF + D  # ctx | raw e | Z | last-chunk ctx row (host-merged)
EXP_BIAS = -20.0  # constant shift inside exp; cancels in softmax

_CACHE = {}
ONES_HOST = np.ones((1, PD), dtype=np.float32)


def _install_axon_shim():
    """Recreate antenv.axon_hooks (absent in this image) and register the
    ctypes NTFF profiling hook so trace=True works under axon."""
    if "antenv.axon_hooks" in sys.modules:
        return
    try:
        import antenv
    except ImportError:
        return
    mod = types.ModuleType("antenv.axon_hooks")
    _state = {"hook": None}
    mod.set_axon_ntff_profile_hook = lambda h: _state.__setitem__("hook", h)
    mod.get_axon_ntff_profile_hook = lambda: _state["hook"]
    sys.modules["antenv.axon_hooks"] = mod
    antenv.axon_hooks = mod
    try:
        from trn_agent_boot.trn_boot import _ntff_profile_via_ctypes

        hook = _ntff_profile_via_ctypes("/opt/axon/libaxon_pjrt.so")
        if hook is not None:
            mod.set_axon_ntff_profile_hook(hook)
    except Exception:
        pass


def _build():
    import concourse.bacc as bacc
    import concourse.mybir as mybir
    from concourse.tile import TileContext
    from concourse import masks

    F32 = mybir.dt.float32
    F32R = mybir.dt.float32r
    AF = mybir.ActivationFunctionType
    ALU = mybir.AluOpType

    nc = bacc.Bacc("TRN2", target_bir_lowering=False)
    vT = nc.declare_dram_parameter("valuesT", [BP, D, S], F32R, isOutput=False)
    w1 = nc.declare_dram_parameter("w1", [HC, PD, DC, PD], F32R, isOutput=False)
    w2 = nc.declare_dram_parameter("w2", [D, H], F32R, isOutput=False)
    qT = nc.declare_dram_parameter("qT", [D, BP], F32R, isOutput=False)
    vv = nc.declare_dram_parameter("v", [H, 1], F32R, isOutput=False)
    ones_in = nc.declare_dram_parameter("ones", [1, PD], F32R, isOutput=False)
    vnat = nc.declare_dram_parameter("vnat", [TS // PD, PD, D], F32R, isOutput=False)
    out = nc.declare_dram_parameter("out", [BP, OUTW], F32, isOutput=True)

    with TileContext(nc) as tc:
        with (
            tc.tile_pool(name="const", bufs=1) as cpool,
            tc.tile_pool(name="vt", bufs=3) as vtpool,
            tc.tile_pool(name="en", bufs=4) as enpool,
            tc.tile_pool(name="erow", bufs=2) as epool,
            tc.tile_pool(name="kp", bufs=3, space="PSUM") as kpool,
            tc.tile_pool(name="sc", bufs=2, space="PSUM") as scpool,
            tc.tile_pool(name="bc", bufs=2, space="PSUM") as bpool,
        ):
            w1_sb = cpool.tile([PD, HC, DC, PD], F32R, tag="w1")
            w2_sb = cpool.tile([PD, DC, H], F32R, tag="w2")
            qt_sb = cpool.tile([PD, DC, BP], F32R, tag="qt")
            v_sb = cpool.tile([PD, HC], F32R, tag="v")
            ones = cpool.tile([1, PD], F32R, tag="ones")
            onesf = cpool.tile([1, PD], F32, tag="onesf")
            ident = cpool.tile([PD, PD], F32, tag="ident")
            qp_sb = cpool.tile([PD, HC, BP], F32, tag="qp")
            qprow_sb = cpool.tile([BP, H], F32, tag="qprow")
            z_sb = cpool.tile([1, BP, NCHUNK], F32, tag="z")
            zt_sb = cpool.tile([1, BP], F32, tag="zt")
            rz_sb = cpool.tile([1, BP], F32, tag="rz")
            rzc_sb = cpool.tile([PD, BP], F32, tag="rzc")
            ctx_sb = cpool.tile([PD, BP, DC, NCHUNK], F32, tag="ctx")
            ctxf_sb = cpool.tile([PD, BP, DC], F32, tag="ctxf")
            ctxs_sb = cpool.tile([PD, BP, DC], F32, tag="ctxs")
            ct_sb = cpool.tile([DC, BP, PD], F32, tag="ct")
            dump = cpool.tile([PD, TS], F32, tag="dump")
            ebias = cpool.tile([1, 1], F32, tag="ebias")
            ecol_sb = cpool.tile([PD, TS // PD], F32R, tag="ecol")
            crow_sb = cpool.tile([1, D], F32, tag="crow")

            nc.vector.memset(onesf[:], 1.0)
            nc.vector.memset(ebias[:], EXP_BIAS)
            masks.make_identity(nc, ident[:])
            vt_tiles = {}
            vt00 = vtpool.tile([PD, DC, SBLK], F32R, tag="vt", name="vt")
            vt_tiles[(0, 0)] = vt00
            # DMA issue order tracks first consumption: w2/qT feed qp, then
            # the first keys group needs vt00's st0 half and w1's hc0 slab
            # (w1 arrives pre-shuffled hc-major so one 512KB DMA per hc).
            def _vt_dma(tile, dc, bi_blk, c0, c1):
                eng = nc.sync
                bi, blk = bi_blk if isinstance(bi_blk, tuple) else (0, 0)
                eng.dma_start(
                    out=tile[:, dc, c0:c1],
                    in_=vT[bi, dc * PD:(dc + 1) * PD, blk * SBLK + c0:blk * SBLK + c1],
                )

            nc.sync.dma_start(
                out=qt_sb[:],
                in_=qT.rearrange("(c p) b -> p c b", p=PD),
            )
            for dc in range(DC):
                nc.sync.dma_start(out=w2_sb[:, dc, 0:TS], in_=w2[dc * PD:(dc + 1) * PD, 0:TS])
            for dc in range(DC):
                _vt_dma(vt00, dc, 0, 0, TS)
            nc.sync.dma_start(out=w1_sb[:, 0, :, :], in_=w1[0])
            nc.sync.dma_start(out=w1_sb[:, 1, :, :], in_=w1[1])
            for dc in range(DC):
                nc.sync.dma_start(out=w2_sb[:, dc, TS:H], in_=w2[dc * PD:(dc + 1) * PD, TS:H])
                if dc == 1:
                    nc.sync.dma_start(out=w1_sb[:, 2, :, :], in_=w1[2])
                if dc == 3:
                    nc.sync.dma_start(out=w1_sb[:, 3, :, :], in_=w1[3])
                if dc == 5:
                    nc.sync.dma_start(out=w1_sb[:, 4, :, :], in_=w1[4])
                if dc == 7:
                    nc.sync.dma_start(out=ones[:], in_=ones_in[:])
                    nc.sync.dma_start(
                        out=v_sb[:], in_=vv.rearrange("(c p) o -> p (c o)", p=PD)
                    )
            for hc in range(5, HC):
                nc.sync.dma_start(out=w1_sb[:, hc, :, :], in_=w1[hc])
                for dc in range(hc - 5, DC, 3):
                    _vt_dma(vt00, dc, 0, TS, SBLK)

            # query_proj rows via qT as the (4-column) stationary operand,
            # then PE-transpose into bias columns. The second half is
            # deferred into the group loop so the first keys groups run
            # while w2's second half is still arriving.
            HPT = TS // PD  # transposed hc columns per half

            def qp_half(hh):
                qprow_ps = kpool.tile([BP, TS], F32, tag="kp", name="qprow_ps")
                for dc in range(DC):
                    nc.tensor.matmul(
                        qprow_ps[:],
                        qt_sb[:, dc, :],
                        w2_sb[:, dc, hh * TS:(hh + 1) * TS],
                        start=(dc == 0),
                        stop=(dc == DC - 1),
                    )
                nc.vector.tensor_copy(qprow_sb[0:BP, hh * TS:(hh + 1) * TS], qprow_ps[:])
                for hc in range(hh * HPT, (hh + 1) * HPT):
                    qpcol_ps = kpool.tile([PD, BP], F32, tag="kp", name="qpcol_ps")
                    nc.tensor.transpose(
                        qpcol_ps[:], qprow_sb[0:BP, hc * PD:(hc + 1) * PD], ident[0:BP, 0:BP]
                    )
                    nc.vector.tensor_copy(qp_sb[:, hc, :], qpcol_ps[:])

            qp_half(0)

            groups = [
                (bi, blk, st, hc)
                for bi in range(BP)
                for blk in range(NBLK)
                for st in range(NST)
                for hc in range(HC)
            ]
            NG = len(groups)
            en_tiles = {}
            vnat_tiles = []
            sc_tiles = {}
            bc_tiles = {}
            erows = {}
            deferred = {}

            def defer(pos, fn):
                deferred.setdefault(pos, []).append(fn)

            def emit_bc(pos, bj, blkj, stj):
                erow = erows[bj]
                off = blkj * SBLK + stj * TS
                final = bj == BP - 1 and blkj == NBLK - 1 and stj == NST - 1
                if final:
                    # final chunk: PE is idle by now, so contract this
                    # half-block against a natural-layout values slice and
                    # ship the resulting row for the host to merge
                    nc.vector.tensor_reduce(
                        zt_sb[0:1, bj:bj + 1], z_sb[0:1, bj, :],
                        axis=mybir.AxisListType.X, op=ALU.add,
                    )
                    nc.vector.reciprocal(rz_sb[0:1, bj:bj + 1], zt_sb[0:1, bj:bj + 1])
                    nc.sync.dma_start(out=out[bj:bj + 1, D:D + S], in_=erow[0:1, :].bitcast(F32))
                    nc.sync.dma_start(out=out[bj:bj + 1, D + S:D + S + 1], in_=zt_sb[0:1, bj:bj + 1])
                    for k in range(TS // PD):
                        ecol_ps = scpool.tile([PD, 1], F32, tag="sc", name="ecol_ps")
                        nc.tensor.transpose(
                            ecol_ps[:],
                            erow[0:1, off + k * PD:off + (k + 1) * PD].bitcast(F32),
                            onesf[0:1, 0:1],
                        )
                        nc.scalar.copy(ecol_sb[:, k:k + 1], ecol_ps[:])
                    vns = vnat_tiles[0]
                    for dh in range(D // TS):
                        crow_ps = scpool.tile([1, TS], F32, tag="sc", name="crow_ps")
                        for k in range(TS // PD):
                            nc.tensor.matmul(
                                crow_ps[:],
                                ecol_sb[:, k:k + 1],
                                vns[:, k, dh * TS:(dh + 1) * TS],
                                start=(k == 0),
                                stop=(k == TS // PD - 1),
                            )
                        nc.vector.tensor_scalar_mul(
                            crow_sb[0:1, dh * TS:(dh + 1) * TS], crow_ps[:],
                            rz_sb[0:1, bj:bj + 1],
                        )
                    nc.sync.dma_start(
                        out=out[bj:bj + 1, TAILOFF:TAILOFF + D], in_=crow_sb[0:1, :]
                    )
                    defer(pos + 2, lambda: rz_bcast(bj))
                    defer(pos + 4, lambda: ctx_finish(bj))
                    defer(pos + 6, lambda: transpose_out(bj))
                    return
                bc = bpool.tile([PD, TS], F32, tag="bc", name="bc")
                nc.tensor.matmul(
                    bc[:],
                    ones[:],
                    erow[0:1, off:off + TS],
                    start=True,
                    stop=True,
                )
                last = stj == NST - 1 and blkj == NBLK - 1
                if last:
                    # softmax denominator first: keeps the deferred PE
                    # rz-broadcast from stalling behind the ctx chain
                    nc.vector.tensor_reduce(
                        zt_sb[0:1, bj:bj + 1], z_sb[0:1, bj, :],
                        axis=mybir.AxisListType.X, op=ALU.add,
                    )
                    nc.vector.reciprocal(rz_sb[0:1, bj:bj + 1], zt_sb[0:1, bj:bj + 1])
                    nc.sync.dma_start(out=out[bj:bj + 1, D:D + S], in_=erow[0:1, :].bitcast(F32))
                    nc.sync.dma_start(out=out[bj:bj + 1, D + S:D + S + 1], in_=zt_sb[0:1, bj:bj + 1])
                # context partial sums for this half-block on DVE
                vt = vt_tiles[(bj, blkj)]
                for dc in range(DC):
                    nc.vector.tensor_tensor(
                        out=dump[:],
                        in0=vt[:, dc, stj * TS:(stj + 1) * TS].bitcast(F32),
                        in1=bc[:],
                        op=ALU.mult,
                    )
                    nc.vector.tensor_reduce(
                        ctx_sb[:, bj, dc, blkj * NST + stj:blkj * NST + stj + 1],
                        dump[:],
                        axis=mybir.AxisListType.X,
                        op=ALU.add,
                    )
                if last:
                    defer(pos + 2, lambda: rz_bcast(bj))
                    defer(pos + 8, lambda: ctx_finish(bj))
                    defer(pos + 12, lambda: transpose_out(bj))

            def rz_bcast(bj):
                rz_ps = scpool.tile([PD, 1], F32, tag="sc", name="rz_ps")
                nc.tensor.matmul(
                    rz_ps[:], onesf[:], rz_sb[0:1, bj:bj + 1], start=True, stop=True
                )
                nc.vector.tensor_copy(rzc_sb[:, bj:bj + 1], rz_ps[:])

            def ctx_finish(bj):
                nparts = NCHUNK - 1 if bj == BP - 1 else NCHUNK
                nc.vector.tensor_reduce(
                    ctxf_sb[:, bj, :], ctx_sb[:, bj, :, 0:nparts],
                    axis=mybir.AxisListType.X, op=ALU.add,
                )
                nc.vector.tensor_scalar_mul(
                    ctxs_sb[:, bj, :], ctxf_sb[:, bj, :], rzc_sb[:, bj:bj + 1]
                )

            def transpose_out(bj):
                ct_ps = scpool.tile([DC, PD], F32, tag="sc", name="ct_ps")
                nc.tensor.transpose(ct_ps[:], ctxs_sb[:, bj, :], ident[:])
                nc.vector.tensor_copy(ct_sb[:, bj, :], ct_ps[:])
                nc.sync.dma_start(
                    out=out[bj:bj + 1, 0:D].rearrange("a (b c) -> (a b) c", b=DC),
                    in_=ct_sb[:, bj, :],
                )

            def emit_vdot(pos, j):
                bj, blkj, stj, hcj = groups[j]
                scj = sc_tiles[(bj, blkj, stj)]
                nc.tensor.matmul(
                    scj[:],
                    v_sb[:, hcj:hcj + 1],
                    en_tiles.pop(j)[:],
                    start=(hcj == 0),
                    stop=(hcj == HC - 1),
                )
                if hcj == HC - 1:
                    # chunk finished: exp with Z-partial via accum_out
                    if (blkj, stj) == (0, 0):
                        erows[bj] = epool.tile([1, S], F32R, tag="erow", name="erow")
                    erow = erows[bj]
                    c = blkj * NST + stj
                    off = blkj * SBLK + stj * TS
                    nc.scalar.activation(
                        erow[0:1, off:off + TS],
                        scj[:],
                        AF.Exp,
                        bias=ebias[0:1, :],
                        scale=1.0,
                        accum_out=z_sb[0:1, bj, c:c + 1],
                    )
                    defer(pos + 1, lambda: emit_bc(pos + 1, bj, blkj, stj))

            for i, (bi, blk, st, hc) in enumerate(groups):
                if st == 0 and hc == 0 and (bi, blk) not in vt_tiles:
                    vt = vtpool.tile([PD, DC, SBLK], F32R, tag="vt", name="vt")
                    vt_tiles[(bi, blk)] = vt
                    for dc in range(DC):
                        _vt_dma(vt, dc, (bi, blk), 0, SBLK)
                    if (bi, blk) == (BP - 1, NBLK - 1):
                        vnat_big = vtpool.tile([PD, DC, SBLK], F32R, tag="vt", name="vnat")
                        vnat_sb = vnat_big[:, 0:TS // PD, :]
                        vnat_tiles.append(vnat_sb)
                        for k in range(TS // PD):
                            nc.sync.dma_start(out=vnat_sb[:, k, :], in_=vnat[k])
                if hc == 0:
                    sc_tiles[(bi, blk, st)] = scpool.tile([1, TS], F32, tag="sc", name="sc")
                vt = vt_tiles[(bi, blk)]
                kp = kpool.tile([PD, TS], F32, tag="kp", name="kp")
                for dc in range(DC):
                    nc.tensor.matmul(
                        kp[:],
                        w1_sb[:, hc, dc, :],
                        vt[:, dc, st * TS:(st + 1) * TS],
                        start=(dc == 0),
                        stop=(dc == DC - 1),
                    )
                en = enpool.tile([PD, TS], F32R, tag="en", name="en")
                en_tiles[i] = en
                nc.scalar.activation(
                    en[:], kp[:], AF.Tanh,
                    bias=qp_sb[:, hc, bi:bi + 1], scale=1.0,
                )
                if i == 2:
                    qp_half(1)
                for fn in deferred.pop(i, []):
                    fn()
                if i >= 2:
                    emit_vdot(i, i - 2)
                if i == NG - 1:
                    emit_vdot(i, i - 1)
                    emit_vdot(i, i)
                    while deferred:
                        j = min(deferred)
                        for fn in deferred.pop(j):
                            fn()

    nc.finalize()
    return nc


def _get_nc():
    if "nc" not in _CACHE:
        _install_axon_shim()
        _CACHE["nc"] = _build()
    return _CACHE["nc"]


def _shard_inputs(query, values, W1, W2, v):
    q = np.asarray(query, dtype=np.float32)
    vals = np.asarray(values, dtype=np.float32)
    w1 = np.asarray(W1, dtype=np.float32)
    w1 = np.ascontiguousarray(
        w1.reshape(DC, PD, HC, PD).transpose(2, 1, 0, 3)
    )
    w2 = np.ascontiguousarray(np.asarray(W2, dtype=np.float32))
    vcol = np.ascontiguousarray(np.asarray(v, dtype=np.float32).reshape(H, 1))
    in_maps = []
    for i in range(NCORES):
        sl = slice(i * BP, (i + 1) * BP)
        in_maps.append({
            "valuesT": np.ascontiguousarray(vals[sl].transpose(0, 2, 1)),
            "w1": w1,
            "w2": w2,
            "qT": np.ascontiguousarray(q[sl].T),
            "v": vcol,
            "ones": ONES_HOST,
            "vnat": np.ascontiguousarray(
                vals[i * BP + BP - 1, S - TS:S, :].reshape(TS // PD, PD, D)
            ),
        })
    return in_maps


def _run(inputs, trace=False):
    from concourse.bass_utils import run_bass_kernel_spmd

    nc = _get_nc()
    in_maps = _shard_inputs(**inputs)
    res = run_bass_kernel_spmd(nc, in_maps, list(range(NCORES)), trace=trace)
    full = np.concatenate([res.results[i]["out"] for i in range(NCORES)], axis=0)
    context = full[:, :D].copy()
    context[BP - 1::BP] += full[BP - 1::BP, TAILOFF:TAILOFF + D]
    attn = full[:, D:D + S] / full[:, D + S:D + S + 1]
    return (
        np.ascontiguousarray(context.astype(np.float32)),
        np.ascontiguousarray(attn.astype(np.float32)),
    ), res


def kernel(query, values, W1, W2, v):
    (context, attn), _ = _run(
        dict(query=query, values=values, W1=W1, W2=W2, v=v), trace=False
    )
    return context, attn
